# revision 10
# baseline (speedup 1.0000x reference)
"""AdaLoRA routed-LoRA kernel for 8 Trainium2 NeuronCores.

Problem (nn_AdaLoRA): per token t with expert index i:
    ds[t, :]  = slots[t, :] @ down_table[i]            # [1024] @ [1024, 16]
    out[t, :] = (ds[t, :] @ up_table[i]) / sqrt(16)    # [16] @ [16, 1024]

Sharding: data-parallel over batch (B=8 -> one batch row per core; LoRA
tables replicated on every core). Per core: 256 tokens = 2 tiles of 128
tokens (tokens on SBUF partitions). ~32MB of table gather per core; the
kernel targets the DMA roofline with compute hidden under the gather.

Down projection (DVE): indirect-DMA gather each token's 64KB down row
into its partition (two 32KB chunks), then per rank r a fused
scalar_tensor_tensor(mult, mult) with accum_out reduces
slots[t,:]*down_i[:,r] in one pass.

Up projection (TensorEngine): tokens are processed in groups of 8; for
group g a [128,128] @ [128,1024] matmul contracts k=(j,r) against a
block-diagonal lhsT holding ds values (built on-chip from ds via
TensorE transpose + a replicate matmul + affine_select masks), with
rhs = the 8 tokens' up tables gathered as 16 rows each via
host-precomputed indices idx*16+r. All 16 group matmuls accumulate into
one PSUM tile (wrong-token columns are zero). f16 matmul inputs, f32
PSUM accumulation. The 1/sqrt(16) scale folds into the PSUM->SBUF copy
on the scalar engine.
"""

import numpy as np

B, K, DIM, RANK, NE = 8, 256, 1024, 16, 4096
ROW = DIM * RANK  # 16384 elements per down-table row
SCALE = 1.0 / 4.0  # 1/sqrt(RANK)
P = 128
N_TILE = K // P  # 2 token tiles per core
DCH = 2  # down-table chunks per tile (512 d-values each)
GRP = P // 8  # 16 groups of 8 tokens per tile
N_CORES = 8

_CACHE = {}


def _build():
    from concourse import bacc, bass, mybir, tile

    f32 = mybir.dt.float32
    f16 = mybir.dt.float16
    bf16 = mybir.dt.bfloat16
    i32 = mybir.dt.int32
    mult = mybir.AluOpType.mult
    add = mybir.AluOpType.add
    is_equal = mybir.AluOpType.is_equal

    nc = bacc.Bacc("TRN2", target_bir_lowering=False)
    slots = nc.declare_dram_parameter("slots", [K, DIM], f32, isOutput=False)
    idx = nc.declare_dram_parameter("idx", [K, 1], i32, isOutput=False)
    idx16 = nc.declare_dram_parameter("idx16", [K * RANK, 1], i32, isOutput=False)
    down = nc.declare_dram_parameter("down", [NE, ROW], f32, isOutput=False)
    up16 = nc.declare_dram_parameter("up16", [NE * RANK, DIM], f32, isOutput=False)
    out = nc.declare_dram_parameter("out", [K, DIM], f32, isOutput=True)

    DC = DIM // DCH  # 512 d per down chunk

    with tile.TileContext(nc) as tc:
        with (
            tc.tile_pool(name="io", bufs=2) as io_pool,
            tc.tile_pool(name="gather", bufs=2) as gpool,
            tc.tile_pool(name="upg", bufs=4) as upool,
            tc.tile_pool(name="upg16", bufs=24) as upool16,
            tc.tile_pool(name="misc", bufs=1) as mpool,
            tc.tile_pool(name="ps", bufs=2, space="PSUM") as pspool,
            tc.tile_pool(name="psout", bufs=2, space="PSUM") as pspool_out,
        ):
            scratch = mpool.tile([P, DC], f32)

            # ---- one-time constants (affine_select keeps in_ where the
            # affine expr `base + cm*p + pattern.x` satisfies compare_op
            # vs 0, else writes fill) ----
            ident = mpool.tile([P, P], f16)
            nc.gpsimd.memset(ident[:], 1.0)
            nc.gpsimd.affine_select(
                out=ident[:], in_=ident[:], compare_op=is_equal, fill=0.0,
                base=0, channel_multiplier=-1, pattern=[[1, P]],
            )
            E = mpool.tile([RANK, P], f16)  # E[q, x] = (x % 16 == q)
            nc.gpsimd.memset(E[:], 1.0)
            nc.gpsimd.affine_select(
                out=E[:].rearrange("q (xb xi) -> q xb xi", xi=16),
                in_=E[:].rearrange("q (xb xi) -> q xb xi", xi=16),
                compare_op=is_equal, fill=0.0,
                base=0, channel_multiplier=-1, pattern=[[0, 8], [1, 16]],
            )
            A = mpool.tile([8, P], f32)  # A[q, x] = (x // 16 == q)
            nc.gpsimd.memset(A[:], 1.0)
            nc.gpsimd.affine_select(
                out=A[:].rearrange("q (xb xi) -> q xb xi", xi=16),
                in_=A[:].rearrange("q (xb xi) -> q xb xi", xi=16),
                compare_op=is_equal, fill=0.0,
                base=0, channel_multiplier=-1, pattern=[[1, 8], [0, 16]],
            )
            Bm = mpool.tile([8, P], f32)  # B[q, t] = (t % 8 == q)
            nc.gpsimd.memset(Bm[:], 1.0)
            nc.gpsimd.affine_select(
                out=Bm[:].rearrange("q (tb tj) -> q tb tj", tj=8),
                in_=Bm[:].rearrange("q (tb tj) -> q tb tj", tj=8),
                compare_op=is_equal, fill=0.0,
                base=0, channel_multiplier=-1, pattern=[[0, 16], [1, 8]],
            )
            M_psum = pspool.tile([P, P], f32, space="PSUM", tag="rep")
            nc.tensor.matmul(out=M_psum[:], lhsT=A[:], rhs=Bm[:], start=True, stop=True)
            M = mpool.tile([P, P], f32)  # M[p, t] = (p//16 == t%8)
            nc.vector.tensor_copy(out=M[:], in_=M_psum[:])

            # 16 zero-padded lhsT buffers: lhsT_all[:, g, :] is zero except
            # columns 8g..8g+8 (refilled per tile; zeros persist)
            lhsT_all = mpool.tile([P, GRP, P], f16)
            nc.vector.memset(lhsT_all[:], 0.0)

            for t in range(N_TILE):
                tok = slice(t * P, (t + 1) * P)
                idx_t = io_pool.tile([P, 1], i32, tag="idx")
                nc.sync.dma_start(out=idx_t[:], in_=idx[tok, :])
                idx16_t = io_pool.tile([P, GRP], i32, tag="idx16")
                nc.sync.dma_start(
                    out=idx16_t[:],
                    in_=idx16[t * P * RANK : (t + 1) * P * RANK, 0].rearrange(
                        "(g p) -> p g", p=P
                    ),
                )
                slots_t = io_pool.tile([P, DIM], f32, tag="slots")
                nc.sync.dma_start(out=slots_t[:], in_=slots[tok, :])
                slots16_t = io_pool.tile([P, DIM], bf16, tag="slots16")
                nc.scalar.copy(out=slots16_t[:], in_=slots_t[:])

                # ---- down projection -> ds [128, 16] f32 ----
                dsp = []
                for c in range(DCH):
                    dch = gpool.tile([P, DC, RANK], f32, tag="dch")
                    nc.gpsimd.indirect_dma_start(
                        out=dch[:].rearrange("p d r -> p (d r)"),
                        out_offset=None,
                        in_=down[:],
                        in_offset=bass.IndirectOffsetOnAxis(ap=idx_t[:, :1], axis=0),
                        element_offset=c * DC * RANK,
                    )
                    dch16 = gpool.tile([P, DC, RANK], bf16, tag="dch16")
                    nc.scalar.copy(out=dch16[:], in_=dch[:])
                    dsp_c = io_pool.tile([P, RANK], f32, tag=f"dsp{c}")
                    for r in range(RANK):
                        nc.vector.scalar_tensor_tensor(
                            out=scratch[:],
                            in0=slots16_t[:, c * DC : (c + 1) * DC],
                            scalar=1.0,
                            in1=dch16[:, :, r],
                            op0=mult,
                            op1=mult,
                            accum_out=dsp_c[:, r : r + 1],
                        )
                    dsp.append(dsp_c)
                ds16 = io_pool.tile([P, RANK], f16, tag="ds16")
                nc.vector.tensor_tensor(
                    out=ds16[:], in0=dsp[0][:], in1=dsp[1][:], op=add
                )

                # ---- build block-diagonal lhsT from ds ----
                dsT_psum = pspool.tile([RANK, P], f16, space="PSUM", tag="dsT")
                nc.tensor.transpose(out=dsT_psum[:], in_=ds16[:], identity=ident[:])
                dsT = io_pool.tile([RANK, P], f16, tag="dsT")
                nc.vector.tensor_copy(out=dsT[:], in_=dsT_psum[:])
                rep_psum = pspool.tile([P, P], f32, space="PSUM", tag="rep")
                nc.tensor.matmul(
                    out=rep_psum[:], lhsT=E[:], rhs=dsT[:], start=True, stop=True
                )
                for g in range(GRP):
                    nc.vector.tensor_tensor(
                        out=lhsT_all[:, g, 8 * g : 8 * g + 8],
                        in0=rep_psum[:, 8 * g : 8 * g + 8],
                        in1=M[:, 8 * g : 8 * g + 8],
                        op=mult,
                    )

                # ---- up projection on TensorE ----
                out_psum = pspool_out.tile([P, DIM], f32, space="PSUM", tag="outp")
                for g in range(GRP):
                    upc = upool.tile([P, DIM], f32, tag="upc")
                    nc.gpsimd.indirect_dma_start(
                        out=upc[:],
                        out_offset=None,
                        in_=up16[:],
                        in_offset=bass.IndirectOffsetOnAxis(
                            ap=idx16_t[:, g : g + 1], axis=0
                        ),
                    )
                    upc16 = upool16.tile([P, DIM], f16, tag="upc16")
                    nc.scalar.copy(out=upc16[:], in_=upc[:])
                    for h in range(2):
                        n0, n1 = h * 512, (h + 1) * 512
                        nc.tensor.matmul(
                            out=out_psum[:, n0:n1],
                            lhsT=lhsT_all[:, g, :],
                            rhs=upc16[:, n0:n1],
                            start=(g == 0),
                            stop=(g == GRP - 1),
                        )
                out_sb = io_pool.tile([P, DIM], f32, tag="osb")
                nc.scalar.mul(out_sb[:], out_psum[:], SCALE)
                nc.sync.dma_start(out=out[tok, :], in_=out_sb[:])
    nc.compile()
    return nc


def _get_nc():
    if "nc" not in _CACHE:
        _CACHE["nc"] = _build()
    return _CACHE["nc"]


def _prep_in_maps(slots, indices, down_proj_values, up_proj_values):
    slots = np.ascontiguousarray(np.asarray(slots, dtype=np.float32))
    indices = np.ascontiguousarray(np.asarray(indices).astype(np.int32))
    down = np.ascontiguousarray(
        np.asarray(down_proj_values, dtype=np.float32).reshape(NE, ROW)
    )
    up16 = np.ascontiguousarray(
        np.asarray(up_proj_values, dtype=np.float32).reshape(NE * RANK, DIM)
    )
    assert slots.shape == (B, K, DIM) and indices.shape == (B, K)
    # idx16[t*2048 + g*128 + p] = indices[128*t + 8*g + p//16]*16 + p%16
    # (per tile t, group g of 8 tokens; partition p = (j, r) = (p//16, p%16))
    p = np.arange(P)
    j, r = p // 16, p % 16
    t_i = np.arange(N_TILE)[:, None, None]
    g_i = np.arange(GRP)[None, :, None]
    toks = 128 * t_i + 8 * g_i + j[None, None, :]  # [N_TILE, GRP, P]
    in_maps = []
    for i in range(N_CORES):
        idx16 = (indices[i][toks] * RANK + r[None, None, :]).astype(np.int32)
        in_maps.append(
            {
                "slots": slots[i],
                "idx": indices[i].reshape(K, 1),
                "idx16": idx16.reshape(K * RANK, 1),
                "down": down,
                "up16": up16,
            }
        )
    return in_maps


def _run(in_maps, trace=False):
    from concourse.bass_utils import run_bass_kernel_spmd

    nc = _get_nc()
    return run_bass_kernel_spmd(
        nc, in_maps, core_ids=list(range(N_CORES)), trace=trace
    )


def kernel(slots, indices, down_proj_values, up_proj_values):
    in_maps = _prep_in_maps(slots, indices, down_proj_values, up_proj_values)
    res = _run(in_maps)
    out = np.stack([res.results[i]["out"] for i in range(N_CORES)], axis=0)
    return out.astype(np.float32)


# revision 11
# speedup vs baseline: 1.2459x; 1.2459x over previous
"""AdaLoRA routed-LoRA kernel for 8 Trainium2 NeuronCores.

Problem (nn_AdaLoRA): per token t with expert index i:
    ds[t, :]  = slots[t, :] @ down_table[i]            # [1024] @ [1024, 16]
    out[t, :] = (ds[t, :] @ up_table[i]) / sqrt(16)    # [16] @ [16, 1024]

Sharding: data-parallel over batch (B=8 -> one batch row per core; LoRA
tables replicated on every core). Per core: 256 tokens = 2 tiles of 128
tokens (tokens on SBUF partitions). ~32MB of table gather per core; the
kernel targets the DMA roofline with compute hidden under the gather.

Down projection (DVE): indirect-DMA gather each token's 64KB down row
into its partition (two 32KB chunks), then per rank r a fused
scalar_tensor_tensor(mult, mult) with accum_out reduces
slots[t,:]*down_i[:,r] in one pass.

Up projection (TensorEngine): tokens are processed in groups of 8; for
group g a [128,128] @ [128,1024] matmul contracts k=(j,r) against a
block-diagonal lhsT holding ds values (built on-chip from ds via
TensorE transpose + a replicate matmul + affine_select masks), with
rhs = the 8 tokens' up tables gathered as 16 rows each via
host-precomputed indices idx*16+r. All 16 group matmuls accumulate into
one PSUM tile (wrong-token columns are zero). f16 matmul inputs, f32
PSUM accumulation. The 1/sqrt(16) scale folds into the PSUM->SBUF copy
on the scalar engine.
"""

import numpy as np

B, K, DIM, RANK, NE = 8, 256, 1024, 16, 4096
ROW = DIM * RANK  # 16384 elements per down-table row
SCALE = 1.0 / 4.0  # 1/sqrt(RANK)
P = 128
N_TILE = K // P  # 2 token tiles per core
DCH = 2  # down-table chunks per tile (512 d-values each)
GRP = P // 8  # 16 groups of 8 tokens per tile
N_CORES = 8

_CACHE = {}


def _build():
    from concourse import bacc, bass, mybir, tile

    f32 = mybir.dt.float32
    f16 = mybir.dt.float16
    bf16 = mybir.dt.bfloat16
    i32 = mybir.dt.int32
    mult = mybir.AluOpType.mult
    add = mybir.AluOpType.add
    is_equal = mybir.AluOpType.is_equal

    nc = bacc.Bacc("TRN2", target_bir_lowering=False)
    slots = nc.declare_dram_parameter("slots", [K, DIM], f32, isOutput=False)
    idx = nc.declare_dram_parameter("idx", [K, 1], i32, isOutput=False)
    idx16 = nc.declare_dram_parameter("idx16", [K * RANK, 1], i32, isOutput=False)
    down = nc.declare_dram_parameter("down", [NE, ROW], f32, isOutput=False)
    up16 = nc.declare_dram_parameter("up16", [NE * RANK, DIM], f32, isOutput=False)
    out = nc.declare_dram_parameter("out", [K, DIM], f32, isOutput=True)

    DC = DIM // DCH  # 512 d per down chunk

    with tile.TileContext(nc) as tc:
        with (
            tc.tile_pool(name="io", bufs=2) as io_pool,
            tc.tile_pool(name="gather", bufs=2) as gpool,
            tc.tile_pool(name="upg", bufs=4) as upool,
            tc.tile_pool(name="upg16", bufs=24) as upool16,
            tc.tile_pool(name="misc", bufs=1) as mpool,
            tc.tile_pool(name="ps", bufs=2, space="PSUM") as pspool,
            tc.tile_pool(name="psout", bufs=2, space="PSUM") as pspool_out,
        ):
            scratch = mpool.tile([P, DIM], f32)

            # ---- one-time constants (affine_select keeps in_ where the
            # affine expr `base + cm*p + pattern.x` satisfies compare_op
            # vs 0, else writes fill) ----
            ident = mpool.tile([P, P], f16)
            nc.gpsimd.memset(ident[:], 1.0)
            nc.gpsimd.affine_select(
                out=ident[:], in_=ident[:], compare_op=is_equal, fill=0.0,
                base=0, channel_multiplier=-1, pattern=[[1, P]],
            )
            E = mpool.tile([RANK, P], f16)  # E[q, x] = (x % 16 == q)
            nc.gpsimd.memset(E[:], 1.0)
            nc.gpsimd.affine_select(
                out=E[:].rearrange("q (xb xi) -> q xb xi", xi=16),
                in_=E[:].rearrange("q (xb xi) -> q xb xi", xi=16),
                compare_op=is_equal, fill=0.0,
                base=0, channel_multiplier=-1, pattern=[[0, 8], [1, 16]],
            )
            A = mpool.tile([8, P], f32)  # A[q, x] = (x // 16 == q)
            nc.gpsimd.memset(A[:], 1.0)
            nc.gpsimd.affine_select(
                out=A[:].rearrange("q (xb xi) -> q xb xi", xi=16),
                in_=A[:].rearrange("q (xb xi) -> q xb xi", xi=16),
                compare_op=is_equal, fill=0.0,
                base=0, channel_multiplier=-1, pattern=[[1, 8], [0, 16]],
            )
            Bm = mpool.tile([8, P], f32)  # B[q, t] = (t % 8 == q)
            nc.gpsimd.memset(Bm[:], 1.0)
            nc.gpsimd.affine_select(
                out=Bm[:].rearrange("q (tb tj) -> q tb tj", tj=8),
                in_=Bm[:].rearrange("q (tb tj) -> q tb tj", tj=8),
                compare_op=is_equal, fill=0.0,
                base=0, channel_multiplier=-1, pattern=[[0, 16], [1, 8]],
            )
            M_psum = pspool.tile([P, P], f32, space="PSUM", tag="rep")
            nc.tensor.matmul(out=M_psum[:], lhsT=A[:], rhs=Bm[:], start=True, stop=True)
            M = mpool.tile([P, P], f32)  # M[p, t] = (p//16 == t%8)
            nc.vector.tensor_copy(out=M[:], in_=M_psum[:])

            # 16 zero-padded lhsT buffers: lhsT_all[:, g, :] is zero except
            # columns 8g..8g+8 (refilled per tile; zeros persist)
            lhsT_all = mpool.tile([P, GRP, P], f16)
            nc.vector.memset(lhsT_all[:], 0.0)

            for t in range(N_TILE):
                tok = slice(t * P, (t + 1) * P)
                idx_t = io_pool.tile([P, 1], i32, tag="idx")
                nc.sync.dma_start(out=idx_t[:], in_=idx[tok, :])
                idx16_t = io_pool.tile([P, GRP], i32, tag="idx16")
                nc.sync.dma_start(
                    out=idx16_t[:],
                    in_=idx16[t * P * RANK : (t + 1) * P * RANK, 0].rearrange(
                        "(g p) -> p g", p=P
                    ),
                )
                slots_t = io_pool.tile([P, DIM], f32, tag="slots")
                nc.sync.dma_start(out=slots_t[:], in_=slots[tok, :])

                # ---- down projection -> ds [128, 16] f32 ----
                RC = RANK // DCH  # ranks per down chunk
                ds = io_pool.tile([P, RANK], f32, tag="ds")
                for c in range(DCH):
                    dch = gpool.tile([P, RC, DIM], f32, tag="dch")
                    nc.gpsimd.indirect_dma_start(
                        out=dch[:].rearrange("p r d -> p (r d)"),
                        out_offset=None,
                        in_=down[:],
                        in_offset=bass.IndirectOffsetOnAxis(ap=idx_t[:, :1], axis=0),
                        element_offset=c * RC * DIM,
                    )
                    for rl in range(RC):
                        r = c * RC + rl
                        nc.vector.scalar_tensor_tensor(
                            out=scratch[:],
                            in0=slots_t[:],
                            scalar=1.0,
                            in1=dch[:, rl, :],
                            op0=mult,
                            op1=mult,
                            accum_out=ds[:, r : r + 1],
                        )
                ds16 = io_pool.tile([P, RANK], f16, tag="ds16")
                nc.vector.tensor_copy(out=ds16[:], in_=ds[:])

                # ---- build block-diagonal lhsT from ds ----
                dsT_psum = pspool.tile([RANK, P], f16, space="PSUM", tag="dsT")
                nc.tensor.transpose(out=dsT_psum[:], in_=ds16[:], identity=ident[:])
                dsT = io_pool.tile([RANK, P], f16, tag="dsT")
                nc.vector.tensor_copy(out=dsT[:], in_=dsT_psum[:])
                rep_psum = pspool.tile([P, P], f32, space="PSUM", tag="rep")
                nc.tensor.matmul(
                    out=rep_psum[:], lhsT=E[:], rhs=dsT[:], start=True, stop=True
                )
                for g in range(GRP):
                    nc.vector.tensor_tensor(
                        out=lhsT_all[:, g, 8 * g : 8 * g + 8],
                        in0=rep_psum[:, 8 * g : 8 * g + 8],
                        in1=M[:, 8 * g : 8 * g + 8],
                        op=mult,
                    )

                # ---- up projection on TensorE ----
                out_psum = pspool_out.tile([P, DIM], f32, space="PSUM", tag="outp")
                for g in range(GRP):
                    upc = upool.tile([P, DIM], f32, tag="upc")
                    nc.gpsimd.indirect_dma_start(
                        out=upc[:],
                        out_offset=None,
                        in_=up16[:],
                        in_offset=bass.IndirectOffsetOnAxis(
                            ap=idx16_t[:, g : g + 1], axis=0
                        ),
                    )
                    upc16 = upool16.tile([P, DIM], f16, tag="upc16")
                    nc.scalar.copy(out=upc16[:], in_=upc[:])
                    for h in range(2):
                        n0, n1 = h * 512, (h + 1) * 512
                        nc.tensor.matmul(
                            out=out_psum[:, n0:n1],
                            lhsT=lhsT_all[:, g, :],
                            rhs=upc16[:, n0:n1],
                            start=(g == 0),
                            stop=(g == GRP - 1),
                        )
                out_sb = io_pool.tile([P, DIM], f32, tag="osb")
                nc.scalar.mul(out_sb[:], out_psum[:], SCALE)
                nc.sync.dma_start(out=out[tok, :], in_=out_sb[:])
    nc.compile()
    return nc


def _get_nc():
    if "nc" not in _CACHE:
        _CACHE["nc"] = _build()
    return _CACHE["nc"]


def _prep_in_maps(slots, indices, down_proj_values, up_proj_values):
    slots = np.ascontiguousarray(np.asarray(slots, dtype=np.float32))
    indices = np.ascontiguousarray(np.asarray(indices).astype(np.int32))
    down = np.ascontiguousarray(
        np.asarray(down_proj_values, dtype=np.float32).transpose(0, 2, 1).reshape(NE, ROW)
    )
    up16 = np.ascontiguousarray(
        np.asarray(up_proj_values, dtype=np.float32).reshape(NE * RANK, DIM)
    )
    assert slots.shape == (B, K, DIM) and indices.shape == (B, K)
    # idx16[t*2048 + g*128 + p] = indices[128*t + 8*g + p//16]*16 + p%16
    # (per tile t, group g of 8 tokens; partition p = (j, r) = (p//16, p%16))
    p = np.arange(P)
    j, r = p // 16, p % 16
    t_i = np.arange(N_TILE)[:, None, None]
    g_i = np.arange(GRP)[None, :, None]
    toks = 128 * t_i + 8 * g_i + j[None, None, :]  # [N_TILE, GRP, P]
    in_maps = []
    for i in range(N_CORES):
        idx16 = (indices[i][toks] * RANK + r[None, None, :]).astype(np.int32)
        in_maps.append(
            {
                "slots": slots[i],
                "idx": indices[i].reshape(K, 1),
                "idx16": idx16.reshape(K * RANK, 1),
                "down": down,
                "up16": up16,
            }
        )
    return in_maps


def _run(in_maps, trace=False):
    from concourse.bass_utils import run_bass_kernel_spmd

    nc = _get_nc()
    return run_bass_kernel_spmd(
        nc, in_maps, core_ids=list(range(N_CORES)), trace=trace
    )


def kernel(slots, indices, down_proj_values, up_proj_values):
    in_maps = _prep_in_maps(slots, indices, down_proj_values, up_proj_values)
    res = _run(in_maps)
    out = np.stack([res.results[i]["out"] for i in range(N_CORES)], axis=0)
    return out.astype(np.float32)
